# revision 6
# baseline (speedup 1.0000x reference)
"""Trainium2 Bass kernel for a pre-LN transformer decoder block.

Problem: x:[2,2048,1024] f32, causal mask, 16 heads, DFF=4096.
  out = x + Attn(LN1(x)); out = out + FFN(LN2(out))

Strategy (8 NeuronCores, collective-free SPMD):
  - Core c handles batch c//4, query rows [(c%4)*512, (c%4)*512+512).
  - Each core redundantly computes LN1 + K/V over its batch's full 2048
    context rows (uniform program; per-core differences live only in the
    input data: x slices and the causal mask).
  - All activations are kept feature-major ([feature-part, token-free]) so
    no activation transposes are needed anywhere except V (done on PE).
  - Softmax without max-subtraction (scores are provably tiny: |s/8|<4),
    multiplicative 0/1 causal mask applied post-exp, row-sums obtained for
    free via a ones-column appended to V.
  - Matmuls in bf16 (fp32 accumulate in PSUM); LN stats in fp32.
"""

import sys
import contextlib
import numpy as np

for _p in ("/opt/trn_rl_repo", "/root/.axon_site/_ro/trn_rl_repo"):
    if _p not in sys.path:
        sys.path.insert(0, _p)

import ml_dtypes  # noqa: E402
import concourse.bass as bass  # noqa: E402
import concourse.mybir as mybir  # noqa: E402
import concourse.tile as tile  # noqa: E402
from concourse import bacc  # noqa: E402
from concourse.bass_utils import run_bass_kernel_spmd  # noqa: E402
from concourse.masks import make_identity  # noqa: E402

P = 128
DH = 64
EPS = 1e-5
BF16 = mybir.dt.bfloat16
F32 = mybir.dt.float32
AF = mybir.ActivationFunctionType

FULL_CFG = dict(B=2, S=2048, D=1024, H=16, DFF=4096, N_CORES=8)

_PROG_CACHE = {}


def _build_program(S, D, H, DFF, TQ, n_iter=1):
    """One-core SPMD program: full decoder block for TQ query rows with an
    S-row context. All cores run this identical program on different data."""
    E = H * DH
    assert E == D
    J = D // P          # feature 128-tiles
    JF = DFF // P
    KC = S // P         # context 128-chunks (k dim of attention)
    NT = S // 512       # context 512-chunks
    HPJ = P // DH       # heads per 128-tile (=2)
    assert TQ <= 512

    nc = bacc.Bacc(None, target_bir_lowering=False)

    # ---- I/O ----
    xcTb = nc.dram_tensor("xcTb", [D, S], BF16, kind="ExternalInput")
    xqT = nc.dram_tensor("xqT", [D, TQ], F32, kind="ExternalInput")
    maskT = nc.dram_tensor("maskT", [S, TQ], BF16, kind="ExternalInput")
    wqT = nc.dram_tensor("wqT", [D, E], BF16, kind="ExternalInput")
    wkT = nc.dram_tensor("wkT", [D, E], BF16, kind="ExternalInput")
    wvT = nc.dram_tensor("wvT", [D, E], BF16, kind="ExternalInput")
    woT = nc.dram_tensor("woT", [E, D], BF16, kind="ExternalInput")
    w1T = nc.dram_tensor("w1T", [D, DFF], BF16, kind="ExternalInput")
    w2T = nc.dram_tensor("w2T", [DFF, D], BF16, kind="ExternalInput")
    ln1g = nc.dram_tensor("ln1g", [D], F32, kind="ExternalInput")
    ln1b = nc.dram_tensor("ln1b", [D], F32, kind="ExternalInput")
    ln2g = nc.dram_tensor("ln2g", [D], F32, kind="ExternalInput")
    ln2b = nc.dram_tensor("ln2b", [D], F32, kind="ExternalInput")
    b1 = nc.dram_tensor("b1", [DFF], F32, kind="ExternalInput")
    b2 = nc.dram_tensor("b2", [D], F32, kind="ExternalInput")
    outT = nc.dram_tensor("outT", [D, TQ], F32, kind="ExternalOutput")

    # j-tiled views ("(j p) t -> p j t")
    xcTb_r = xcTb.rearrange("(j p) t -> p j t", p=P)
    xqT_r = xqT.rearrange("(j p) t -> p j t", p=P)
    maskT_r = maskT.rearrange("(kc p) q -> p kc q", p=P)
    wqT_r = wqT.rearrange("(j p) e -> p j e", p=P)
    wkT_r = wkT.rearrange("(j p) e -> p j e", p=P)
    wvT_r = wvT.rearrange("(j p) e -> p j e", p=P)
    woT_r = woT.rearrange("(j p) e -> p j e", p=P)
    w1T_r = w1T.rearrange("(j p) f -> p j f", p=P)
    w2T_r = w2T.rearrange("(jf p) e -> p jf e", p=P)
    outT_r = outT.rearrange("(j p) q -> p j q", p=P)

    loop_cm = nc.Fori(0, n_iter) if n_iter > 1 else contextlib.nullcontext()
    with loop_cm, tile.TileContext(nc) as tc:
        with (
            tc.tile_pool(name="dram", bufs=1, space="DRAM") as dram,
            tc.tile_pool(name="const", bufs=1) as const,
            tc.tile_pool(name="persist", bufs=1) as persist,
            tc.tile_pool(name="scr", bufs=2) as scr,
            tc.tile_pool(name="scr_s", bufs=2) as scr_s,
            tc.tile_pool(name="ps_misc", bufs=3, space="PSUM") as ps_misc,
            tc.tile_pool(name="ps_mm", bufs=2, space="PSUM") as ps_mm,
        ):
            # DRAM scratch
            ln1d = dram.tile([D, S], BF16)
            ktd = dram.tile([E, S], BF16)
            vtd = dram.tile([E, S], BF16)
            vd = dram.tile([H, S, DH], BF16)
            qtd = dram.tile([H, DH, TQ], BF16)
            ln1d_r = ln1d[:].rearrange("(j p) t -> p j t", p=P)

            # constants
            ones_col = const.tile([P, 1], BF16)
            nc.vector.memset(ones_col[:], 1.0)
            ones_row = const.tile([1, P], BF16)
            nc.vector.memset(ones_row[:], 1.0)
            eps_t = const.tile([1, 1], F32)
            nc.vector.memset(eps_t[:], EPS)
            ident = const.tile([P, P], BF16)
            make_identity(nc, ident[:])
            ln1g_c = const.tile([P, J], F32)
            nc.sync.dma_start(ln1g_c[:], ln1g.rearrange("(j p) -> p j", p=P))
            ln1b_c = const.tile([P, J], F32)
            nc.sync.dma_start(ln1b_c[:], ln1b.rearrange("(j p) -> p j", p=P))
            ln2g_c = const.tile([P, J], F32)
            nc.sync.dma_start(ln2g_c[:], ln2g.rearrange("(j p) -> p j", p=P))
            ln2b_c = const.tile([P, J], F32)
            nc.sync.dma_start(ln2b_c[:], ln2b.rearrange("(j p) -> p j", p=P))
            b1_c = const.tile([P, JF], F32)
            nc.sync.dma_start(b1_c[:], b1.rearrange("(j p) -> p j", p=P))
            b2_c = const.tile([P, J], F32)
            nc.sync.dma_start(b2_c[:], b2.rearrange("(j p) -> p j", p=P))

            # persistent activations
            xq_sb = persist.tile([P, J, TQ], F32)
            nc.sync.dma_start(xq_sb[:], xqT_r)
            attn_sb = persist.tile([P, J, TQ], BF16)
            y_sb = persist.tile([P, J, TQ], F32)

            def ln_stats(src_get, T, n_j):
                """Column LN stats over n_j*P features. src_get(j) -> bf16
                [P, T] tile. Returns (pmu, prs) psum [P, T] broadcasts."""
                ps_x = ps_misc.tile([P, 512], F32, tag="misc")
                ps_q = ps_misc.tile([P, 512], F32, tag="misc")
                for j in range(n_j):
                    xt = src_get(j)
                    sq = scr.tile([P, 512], BF16, tag="sq")
                    nc.vector.tensor_mul(sq[:, :T], xt, xt)
                    nc.tensor.matmul(ps_x[:1, :T], ones_col[:], xt,
                                     start=(j == 0), stop=(j == n_j - 1))
                    nc.tensor.matmul(ps_q[:1, :T], ones_col[:], sq[:, :T],
                                     start=(j == 0), stop=(j == n_j - 1))
                inv_d = 1.0 / (n_j * P)
                mu = scr_s.tile([1, 512], F32, tag="mu")
                nc.scalar.mul(mu[:, :T], ps_x[:1, :T], inv_d)
                ex2 = scr_s.tile([1, 512], F32, tag="ex2")
                nc.scalar.mul(ex2[:, :T], ps_q[:1, :T], inv_d)
                var = scr_s.tile([1, 512], F32, tag="var")
                nc.vector.tensor_mul(var[:, :T], mu[:, :T], mu[:, :T])
                nc.vector.tensor_sub(var[:, :T], ex2[:, :T], var[:, :T])
                nc.scalar.activation(var[:, :T], var[:, :T], AF.Sqrt,
                                     bias=eps_t[:], scale=1.0)
                nc.vector.reciprocal(var[:, :T], var[:, :T])
                mub = scr_s.tile([1, 512], BF16, tag="mub")
                nc.scalar.copy(mub[:, :T], mu[:, :T])
                rsb = scr_s.tile([1, 512], BF16, tag="rsb")
                nc.scalar.copy(rsb[:, :T], var[:, :T])
                pmu = ps_misc.tile([P, 512], F32, tag="misc")
                nc.tensor.matmul(pmu[:, :T], ones_row[:], mub[:, :T],
                                 start=True, stop=True)
                prs = ps_misc.tile([P, 512], F32, tag="misc")
                nc.tensor.matmul(prs[:, :T], ones_row[:], rsb[:, :T],
                                 start=True, stop=True)
                return pmu, prs

            def ln_norm_tile(xt, pmu, prs, g_c, b_c, j, T, out_ap):
                """out = (xt - mu) * rstd * g + b   (bf16 out)."""
                t1 = scr.tile([P, 512], F32, tag="t1")
                nc.vector.tensor_sub(t1[:, :T], xt, pmu[:, :T])
                nc.vector.tensor_mul(t1[:, :T], t1[:, :T], prs[:, :T])
                nc.scalar.activation(out_ap, t1[:, :T], AF.Identity,
                                     bias=b_c[:, j:j + 1], scale=g_c[:, j:j + 1])

            # ---------------- Phase 1: LN1 over context -> ln1d ----------------
            for t in range(NT):
                ts = slice(t * 512, (t + 1) * 512)
                xts = []

                def get_ctx(j, ts=ts, xts=xts):
                    xt = scr.tile([P, 512], BF16, tag="xt")
                    nc.sync.dma_start(xt[:], xcTb_r[:, j, ts])
                    xts.append(xt)
                    return xt[:, :]

                pmu, prs = ln_stats(get_ctx, 512, J)
                for j in range(J):
                    xt2 = scr.tile([P, 512], BF16, tag="xt2")
                    nc.sync.dma_start(xt2[:], xcTb_r[:, j, ts])
                    lo = scr.tile([P, 512], BF16, tag="lo")
                    ln_norm_tile(xt2[:, :], pmu, prs, ln1g_c, ln1b_c, j,
                                 512, lo[:, :])
                    nc.sync.dma_start(ln1d_r[:, j, ts], lo[:])

            with tc.tile_pool(name="s2", bufs=1) as s2, \
                 tc.tile_pool(name="s2w", bufs=2) as s2w, \
                 tc.tile_pool(name="ps_sc", bufs=2, space="PSUM") as ps_sc, \
                 tc.tile_pool(name="ps_av", bufs=1, space="PSUM") as ps_av:

                # ---------- Phase 1b: LN1 over queries -> q_ln1 (sbuf) ----------
                q_ln1 = s2.tile([P, J, TQ], BF16)

                def get_q(j):
                    xt = scr.tile([P, 512], BF16, tag="xt")
                    nc.scalar.copy(xt[:, :TQ], xq_sb[:, j, :])
                    return xt[:, :TQ]

                pmu, prs = ln_stats(get_q, TQ, J)
                for j in range(J):
                    xb = scr.tile([P, 512], BF16, tag="xt2")
                    nc.scalar.copy(xb[:, :TQ], xq_sb[:, j, :])
                    ln_norm_tile(xb[:, :TQ], pmu, prs, ln1g_c, ln1b_c, j,
                                 TQ, q_ln1[:, j, :])

                # ---------- Phase 2: projections ----------
                # K^T and V^T (feature-major, to DRAM), Q^T (sbuf)
                for t in range(NT):
                    ts = slice(t * 512, (t + 1) * 512)
                    rhsT = s2w.tile([P, J, 512], BF16, tag="ln1n")
                    nc.sync.dma_start(rhsT[:], ln1d_r[:, :, ts])
                    for m in range(J):
                        ms = slice(m * P, (m + 1) * P)
                        for w_r, dst in ((wkT_r, ktd), (wvT_r, vtd)):
                            wt = s2w.tile([P, J, P], BF16, tag="w8")
                            nc.sync.dma_start(wt[:], w_r[:, :, ms])
                            ps = ps_mm.tile([P, 512], F32, tag="mm")
                            for j in range(J):
                                nc.tensor.matmul(ps[:], wt[:, j, :],
                                                 rhsT[:, j, :],
                                                 start=(j == 0),
                                                 stop=(j == J - 1))
                            st = scr.tile([P, 512], BF16, tag="kv")
                            nc.scalar.copy(st[:], ps[:])
                            nc.sync.dma_start(dst[ms, ts], st[:])

                for m in range(J):
                    ms = slice(m * P, (m + 1) * P)
                    wt = s2w.tile([P, J, P], BF16, tag="w8")
                    nc.sync.dma_start(wt[:], wqT_r[:, :, ms])
                    ps = ps_mm.tile([P, 512], F32, tag="mm")
                    for j in range(J):
                        nc.tensor.matmul(ps[:, :TQ], wt[:, j, :], q_ln1[:, j, :],
                                         start=(j == 0), stop=(j == J - 1))
                    qstg = scr.tile([P, 512], BF16, tag="qstg")
                    nc.scalar.copy(qstg[:, :TQ], ps[:, :TQ])
                    for hh in range(HPJ):
                        nc.sync.dma_start(
                            qtd[m * HPJ + hh, :, :],
                            qstg[hh * DH:(hh + 1) * DH, :TQ])

                # V^T -> V (PE transpose), vd[h, s, dh]
                for j in range(J):
                    for kc in range(KC):
                        ks = slice(kc * P, (kc + 1) * P)
                        vt_t = s2w.tile([P, P], BF16, tag="vt")
                        nc.sync.dma_start(vt_t[:], vtd[j * P:(j + 1) * P, ks])
                        ps_t = ps_mm.tile([P, P], BF16, tag="mm")
                        nc.tensor.transpose(ps_t[:], vt_t[:], ident[:])
                        stg = scr_s.tile([P, P], BF16, tag="stg")
                        nc.scalar.copy(stg[:], ps_t[:])
                        for hh in range(HPJ):
                            nc.sync.dma_start(
                                vd[j * HPJ + hh, ks, :],
                                stg[:, hh * DH:(hh + 1) * DH])

                # ---------- Phase 3: attention ----------
                mask_sb = s2.tile([P, KC, TQ], BF16)
                nc.sync.dma_start(mask_sb[:], maskT_r)
                exp_sb = s2.tile([P, KC, TQ], BF16)

                for h in range(H):
                    qt_h = s2w.tile([DH, TQ], BF16, tag="qt")
                    nc.sync.dma_start(qt_h[:], qtd[h, :, :])
                    kt_h = s2w.tile([DH, S], BF16, tag="kt")
                    nc.sync.dma_start(kt_h[:], ktd[h * DH:(h + 1) * DH, :])
                    v_h = s2w.tile([P, KC, DH + 1], BF16, tag="vh")
                    nc.sync.dma_start(
                        v_h[:, :, :DH],
                        vd[h].rearrange("(kc p) dh -> p kc dh", p=P))
                    nc.vector.memset(v_h[:, :, DH:DH + 1], 1.0)

                    for kc in range(KC):
                        ps_s = ps_sc.tile([P, 512], F32, tag="sc")
                        nc.tensor.matmul(ps_s[:, :TQ],
                                         kt_h[:, kc * P:(kc + 1) * P],
                                         qt_h[:],
                                         start=True, stop=True)
                        nc.scalar.activation(exp_sb[:, kc, :], ps_s[:, :TQ],
                                             AF.Exp, scale=1.0 / np.sqrt(DH))
                        nc.vector.tensor_mul(exp_sb[:, kc, :], exp_sb[:, kc, :],
                                             mask_sb[:, kc, :])

                    pav = ps_av.tile([P, 512], F32, tag="av")
                    for kc in range(KC):
                        nc.tensor.matmul(pav[:DH + 1, :TQ], v_h[:, kc, :],
                                         exp_sb[:, kc, :],
                                         start=(kc == 0), stop=(kc == KC - 1))
                    zr = scr_s.tile([1, 512], F32, tag="zr")
                    nc.vector.reciprocal(zr[:, :TQ], pav[DH:DH + 1, :TQ])
                    zrb = scr_s.tile([1, 512], BF16, tag="zrb")
                    nc.scalar.copy(zrb[:, :TQ], zr[:, :TQ])
                    ps_z = ps_misc.tile([P, 512], F32, tag="misc")
                    nc.tensor.matmul(ps_z[:DH, :TQ], ones_row[:, :DH],
                                     zrb[:, :TQ], start=True, stop=True)
                    zb = scr_s.tile([DH, 512], F32, tag="zb")
                    nc.scalar.copy(zb[:, :TQ], ps_z[:DH, :TQ])
                    stg64 = scr_s.tile([DH, 512], BF16, tag="stg64")
                    nc.vector.tensor_mul(stg64[:, :TQ],
                                         pav[:DH, :TQ], zb[:, :TQ])
                    nc.sync.dma_start(
                        attn_sb[(h % HPJ) * DH:(h % HPJ) * DH + DH,
                                h // HPJ, :],
                        stg64[:, :TQ])

                # ---------- Phase 4: Wo + residual -> y ----------
                for m in range(J):
                    ms = slice(m * P, (m + 1) * P)
                    wt = s2w.tile([P, J, P], BF16, tag="w8")
                    nc.sync.dma_start(wt[:], woT_r[:, :, ms])
                    ps = ps_mm.tile([P, 512], F32, tag="mm")
                    for j in range(J):
                        nc.tensor.matmul(ps[:, :TQ], wt[:, j, :], attn_sb[:, j, :],
                                         start=(j == 0), stop=(j == J - 1))
                    nc.vector.tensor_add(y_sb[:, m, :], ps[:, :TQ],
                                         xq_sb[:, m, :])

            # ---------- Phase 5: LN2 -> ln2 (sbuf) ----------
            with tc.tile_pool(name="s3", bufs=1) as s3, \
                 tc.tile_pool(name="s3w", bufs=3) as s3w:
                ln2_sb = s3.tile([P, J, TQ], BF16)

                def get_y(j):
                    xt = scr.tile([P, 512], BF16, tag="xt")
                    nc.scalar.copy(xt[:, :TQ], y_sb[:, j, :])
                    return xt[:, :TQ]

                pmu, prs = ln_stats(get_y, TQ, J)
                for j in range(J):
                    yb = scr.tile([P, 512], BF16, tag="xt2")
                    nc.scalar.copy(yb[:, :TQ], y_sb[:, j, :])
                    ln_norm_tile(yb[:, :TQ], pmu, prs, ln2g_c, ln2b_c, j,
                                 TQ, ln2_sb[:, j, :])

                # ---------- Phase 6: FFN ----------
                h1 = s3.tile([P, JF, TQ], BF16)
                for mf in range(JF):
                    ms = slice(mf * P, (mf + 1) * P)
                    wt = s3w.tile([P, J, P], BF16, tag="w8")
                    nc.sync.dma_start(wt[:], w1T_r[:, :, ms])
                    ps = ps_mm.tile([P, 512], F32, tag="mm")
                    for j in range(J):
                        nc.tensor.matmul(ps[:, :TQ], wt[:, j, :], ln2_sb[:, j, :],
                                         start=(j == 0), stop=(j == J - 1))
                    nc.scalar.activation(h1[:, mf, :], ps[:, :TQ], AF.Relu,
                                         bias=b1_c[:, mf:mf + 1], scale=1.0)

                for m in range(J):
                    ms = slice(m * P, (m + 1) * P)
                    wt = s3w.tile([P, JF, P], BF16, tag="w32")
                    nc.sync.dma_start(wt[:], w2T_r[:, :, ms])
                    ps = ps_mm.tile([P, 512], F32, tag="mm")
                    for jf in range(JF):
                        nc.tensor.matmul(ps[:, :TQ], wt[:, jf, :], h1[:, jf, :],
                                         start=(jf == 0), stop=(jf == JF - 1))
                    t3 = scr.tile([P, 512], F32, tag="t1")
                    nc.vector.tensor_add(t3[:, :TQ], ps[:, :TQ], y_sb[:, m, :])
                    ot = scr.tile([P, 512], F32, tag="ot")
                    nc.scalar.activation(ot[:, :TQ], t3[:, :TQ], AF.Identity,
                                         bias=b2_c[:, m:m + 1], scale=1.0)
                    nc.sync.dma_start(outT_r[:, m, :], ot[:, :TQ])

    nc.compile()
    return nc


def _get_program(S, D, H, DFF, TQ, n_iter=1):
    key = (S, D, H, DFF, TQ, n_iter)
    if key not in _PROG_CACHE:
        _PROG_CACHE[key] = _build_program(S, D, H, DFF, TQ, n_iter)
    return _PROG_CACHE[key]


def _run(x, mask, ln1_g, ln1_b, Wq, Wk, Wv, Wo, ln2_g, ln2_b, W1, b1, W2, b2,
         n_cores, trace=False, n_iter=1):
    B, S, D = x.shape
    DFF = W1.shape[0]
    H = D // DH
    cores_per_b = n_cores // B
    TQ = S // cores_per_b

    nc = _get_program(S, D, H, DFF, TQ, n_iter)

    bf = ml_dtypes.bfloat16
    f32 = np.float32
    wqT = np.ascontiguousarray(np.asarray(Wq, f32).T).astype(bf)
    wkT = np.ascontiguousarray(np.asarray(Wk, f32).T).astype(bf)
    wvT = np.ascontiguousarray(np.asarray(Wv, f32).T).astype(bf)
    woT = np.ascontiguousarray(np.asarray(Wo, f32).T).astype(bf)
    w1T = np.ascontiguousarray(np.asarray(W1, f32).T).astype(bf)
    w2T = np.ascontiguousarray(np.asarray(W2, f32).T).astype(bf)
    shared = dict(
        wqT=wqT, wkT=wkT, wvT=wvT, woT=woT, w1T=w1T, w2T=w2T,
        ln1g=np.asarray(ln1_g, f32), ln1b=np.asarray(ln1_b, f32),
        ln2g=np.asarray(ln2_g, f32), ln2b=np.asarray(ln2_b, f32),
        b1=np.asarray(b1, f32), b2=np.asarray(b2, f32),
    )
    mask2d = np.asarray(mask).reshape(S, S)  # [q, k] bool
    x = np.asarray(x, f32)

    in_maps = []
    for c in range(n_cores):
        b = c // cores_per_b
        q0 = (c % cores_per_b) * TQ
        xcTb = np.ascontiguousarray(x[b].T).astype(bf)
        xqT = np.ascontiguousarray(x[b, q0:q0 + TQ].T)
        mT = np.ascontiguousarray(
            mask2d[q0:q0 + TQ, :].T.astype(f32)).astype(bf)
        in_maps.append(dict(shared, xcTb=xcTb, xqT=xqT, maskT=mT))

    res = run_bass_kernel_spmd(nc, in_maps, list(range(n_cores)), trace=trace)

    out = np.empty((B, S, D), f32)
    for c in range(n_cores):
        b = c // cores_per_b
        q0 = (c % cores_per_b) * TQ
        out[b, q0:q0 + TQ, :] = res.results[c]["outT"].T
    return out, res


def kernel(x, mask, ln1_g, ln1_b, Wq, Wk, Wv, Wo, ln2_g, ln2_b, W1, b1, W2,
           b2):
    out, _ = _run(x, mask, ln1_g, ln1_b, Wq, Wk, Wv, Wo, ln2_g, ln2_b,
                  W1, b1, W2, b2, n_cores=8)
    return out
